# revision 21
# baseline (speedup 1.0000x reference)
"""Multi-head attention TRN2 kernel: 8-core head-sharded tensor parallelism.

Full inputs in, full output out. Each core computes 2 of the 16 heads:
QKV projection (its column slice), flash-style attention, and a partial
out-projection against its row slice of Wo. Host sums the 8 partials + bo.

v3 schedule (bf16 operands, ACT-paced, HAM-warm):
  All matmul operands are bf16: weight loads overlap matmuls (standalone
  LDWEIGHTS + FWL are bf16-only on this stack; f32r matmuls self-load
  weights serially, ~2x slower per MM). PSUM accumulation stays fp32 for
  QKV and ctx; single-shot matmuls (scores, out-proj) write bf16 PSUM so
  a bank holds 1024 outputs.

  lead-in : QKV proj of batch 0 via a 6-bank PSUM pool (N=512 fp32
            accumulation), V-transposes per token block, ACT exp table
            pre-warmed, batch-1 x chunks prefetched.
  slots   : 64 slots (b, qb, kt-pair). Each slot: one [128,2048] bf16
            PSUM score tile = [h0 kt0|kt1 | h1 kt0|kt1] written by four
            row-group-concurrent matmuls, ONE [128,2048] exp ACTIVATE,
            and four ctx matmuls lagged 2 slots.
  overlap : batch-1 QKV (atomic 1-bank closures) + its V transposes are
            interleaved into batch-0 slots; out-projection (one N=1024
            matmul per token tile) is deferred and metered.
  norm    : one [65,512] PSUM->SBUF copy frees the ctx accumulator
            immediately; reciprocal runs at partition 0; PE broadcasts
            the reciprocal row; one tensor-multiply writes ctx2t.
"""
import sys

sys.path.insert(0, "/opt/trn_rl_repo")

from contextlib import ExitStack

import numpy as np
import ml_dtypes

import concourse.bass as bass
import concourse.tile as tile
from concourse import bacc, mybir
from concourse.bass_utils import run_bass_kernel_spmd
from concourse.masks import make_identity

f32 = mybir.dt.float32
bf16 = mybir.dt.bfloat16
EXP = mybir.ActivationFunctionType.Exp
np_bf16 = ml_dtypes.bfloat16

N_CORES = 8
B, S, F = 2, 2048, 1024
H = 16                 # heads total
DK = F // H            # 64
HPC = H // N_CORES     # 2 heads per core
CF = HPC * DK          # 128 = per-core slice of features
T = B * S              # 4096 tokens
NKT = S // 128         # 16 key tiles per sequence
NQB = S // 512         # 4 q-blocks per sequence
NKP = NKT // 2         # 8 kt-pairs per sequence


def build_program(debug_dumps=False):
    nc = bacc.Bacc("TRN2", target_bir_lowering=False, debug=False,
                   num_devices=N_CORES)

    xt_d = nc.dram_tensor("xT", [F, T], bf16, kind="ExternalInput").ap()
    wqkv_d = nc.dram_tensor("Wqkv", [F, 3 * CF], bf16, kind="ExternalInput").ap()
    bqkv_d = nc.dram_tensor("bqkv", [3 * CF, 1], f32, kind="ExternalInput").ap()
    wo_d = nc.dram_tensor("Wo", [CF, F], bf16, kind="ExternalInput").ap()
    yp_d = nc.dram_tensor("yp", [T, F], bf16, kind="ExternalOutput").ap()
    if debug_dumps:
        dbg_qkv_d = nc.dram_tensor("dbg_qkv", [3, 128, T], bf16,
                                   kind="ExternalOutput").ap()
        dbg_vaug_d = nc.dram_tensor("dbg_vaug", [128, B, NKT, HPC, 65], bf16,
                                    kind="ExternalOutput").ap()
        dbg_ctx2t_d = nc.dram_tensor("dbg_ctx2t", [128, B, S], bf16,
                                     kind="ExternalOutput").ap()
        dbg_norm_d = nc.dram_tensor("dbg_norm", [B, NQB, HPC, 2, 512], f32,
                                    kind="ExternalOutput").ap()

    with tile.TileContext(nc) as tc, ExitStack() as ctx:
        const = ctx.enter_context(tc.tile_pool(name="const", bufs=1))
        big = ctx.enter_context(tc.tile_pool(name="big", bufs=1))
        xpool = ctx.enter_context(tc.tile_pool(name="xpool", bufs=3))
        x2pool = ctx.enter_context(tc.tile_pool(name="x2pool", bufs=4))
        etp = ctx.enter_context(tc.tile_pool(name="etp", bufs=4))
        small = ctx.enter_context(tc.tile_pool(name="small", bufs=2))
        ypool = ctx.enter_context(tc.tile_pool(name="ypool", bufs=4))

        # ---- constants ----
        wqkv_sb = const.tile([128, 8, 3 * CF], bf16)  # [f-part, f-tile, col]
        nc.sync.dma_start(wqkv_sb, wqkv_d.rearrange("(a p) n -> p a n", p=128))
        wo_sb = const.tile([128, F], bf16)
        nc.sync.dma_start(wo_sb, wo_d)
        ball = const.tile([128, 3], f32, name="ball")
        nc.sync.dma_start(ball, bqkv_d.rearrange("(c p) o -> p (c o)", p=128))
        btiles = [ball[:, p3:p3 + 1] for p3 in range(3)]
        ident_f = const.tile([128, 128], f32)
        make_identity(nc, ident_f)
        ident_b = const.tile([128, 128], bf16)
        nc.vector.tensor_copy(ident_b, ident_f)
        ones_f = const.tile([128, 64], f32)
        nc.vector.memset(ones_f, 1.0)
        ones_b = const.tile([1, 64], bf16)
        nc.vector.tensor_copy(ones_b, ones_f[0:1, :])

        # warm the ACT exp table early (the ~2.7us table load would
        # otherwise land on the first score slot)
        warm = const.tile([1, 16], f32, name="warm")
        nc.scalar.activation(warm, ones_f[0:1, 0:16], EXP)

        # ---- persistent activations (bf16) ----
        qt_sb = big.tile([128, T], bf16)       # [2 heads x 64 d, tokens]
        kt_sb = big.tile([128, T], bf16)
        vt_sb = big.tile([128, T], bf16)
        vaug_sb = big.tile([128, B, NKT, HPC, 65], bf16)
        ctx2t_sb = big.tile([128, B, S], bf16)  # [2 heads x 64 d, b, tok]

        qkvt = [qt_sb, kt_sb, vt_sb]

        # ones column of vaug for all (b, kt, h) in one strided copy
        nc.vector.tensor_copy(
            vaug_sb[:, :, :, :, 64:65],
            ones_f[:, 0:B * NKT * HPC].rearrange(
                "p (b k h o) -> p b k h o", b=B, k=NKT, h=HPC))

        # prefetch batch-1 x chunks (consumed by the interleaved QKV)
        x2_of = {}

        def dma_x2(tb, split=False):
            for g in range(2):
                x2 = x2pool.tile([128, 4, 1024], bf16, tag="x2",
                                 name=f"x2_{tb}_{g}")
                eng = nc.scalar if (split and g % 2 == 1) else nc.sync
                eng.dma_start(
                    x2,
                    xt_d[g * 512:(g + 1) * 512,
                         tb * 1024:(tb + 1) * 1024].rearrange(
                             "(c p) t -> p c t", p=128))
                for a in range(4):
                    x2_of[(tb, g * 4 + a)] = x2[:, a, :]

        # ================= lead-in: QKV + V-transpose for batch 0 =========
        with tc.tile_pool(name="qkvA", bufs=6, space="PSUM") as qkvA, \
             tc.tile_pool(name="vtA", bufs=2, space="PSUM") as vtA:
            for tb in range(2):          # token blocks 0,1 = batch 0
                pqs = [qkvA.tile([128, 512], f32, tag="pq",
                                 name=f"pq{tb}_{i}") for i in range(6)]
                xts = []
                for g in range(2):       # one DMA per 4 f-chunks
                    xt_t = xpool.tile([128, 4, 1024], bf16, tag="xt",
                                      name=f"xt{tb}_{g}")
                    eng = nc.sync if g % 2 == 0 else nc.scalar
                    eng.dma_start(
                        xt_t,
                        xt_d[g * 512:(g + 1) * 512,
                             tb * 1024:(tb + 1) * 1024].rearrange(
                                 "(c p) t -> p c t", p=128))
                    xts.append(xt_t)
                for a in range(8):
                    for p3 in range(3):
                        for half in range(2):
                            nc.tensor.matmul(
                                pqs[p3 * 2 + half],
                                wqkv_sb[:, a, p3 * CF:(p3 + 1) * CF],
                                xts[a // 4][:, a % 4,
                                            half * 512:(half + 1) * 512],
                                start=(a == 0), stop=(a == 7))
                for p3 in range(3):
                    for half in range(2):
                        dst = qkvt[p3][:, tb * 1024 + half * 512:
                                       tb * 1024 + (half + 1) * 512]
                        nc.vector.tensor_scalar_add(dst, pqs[p3 * 2 + half],
                                                    btiles[p3])
                # V transpose for this token block (8 key tiles)
                for k in range(8):
                    tok = tb * 1024 + k * 128
                    pv = vtA.tile([128, 128], bf16, tag="pv",
                                  name=f"pv{tb}_{k}")
                    nc.tensor.transpose(pv, vt_sb[:, tok:tok + 128], ident_b)
                    nc.vector.tensor_copy(
                        vaug_sb[:, 0, tok // 128, :, 0:64],
                        pv.rearrange("p (h d) -> p h d", h=HPC))
            dma_x2(2, split=True)   # batch-1 first tb prefetch

        # ================= attention era ==================================
        # PSUM budget (16KB/partition = 8 banks):
        #   pss 2x[128,1024]f32 = 4 banks, ctx 2x[65,512]f32 = 2 banks,
        #   pp (qkv-b1 accum / out-proj / norm broadcast / vtrans) 2 banks
        att = ctx.enter_context(tc.tile_pool(name="att", bufs=1, space="PSUM"))

        slots = [(b, qb, kt) for b in range(B) for qb in range(NQB)
                 for kt in range(NKT)]
        NSLOT = len(slots)               # 128

        pss_of = {}
        et_of = {}
        ctx_ps = {}

        def emit_scores(i):
            b, qb, kt = slots[i]
            pss = att.tile([128, 1024], f32, tag="pss", bufs=2,
                           name=f"pss{i}")
            # h0 then h1: adjacent MMs hit different PE row groups AND
            # different PSUM banks -> concurrent
            for h in range(HPC):
                nc.tensor.matmul(
                    pss[:, h * 512:(h + 1) * 512],
                    kt_sb[h * 64:(h + 1) * 64,
                          b * S + kt * 128:b * S + (kt + 1) * 128],
                    qt_sb[h * 64:(h + 1) * 64,
                          b * S + qb * 512:b * S + (qb + 1) * 512],
                    start=True, stop=True)
            pss_of[i] = pss

        def emit_exp(i):
            et = etp.tile([128, 1024], bf16, tag="et", name=f"et{i}")
            nc.scalar.activation(et, pss_of.pop(i), EXP)
            et_of[i] = et

        def emit_ctx(i):
            b, qb, kt = slots[i]
            et = et_of.pop(i)
            for h in range(HPC):
                if kt == 0:
                    ctx_ps[(b, qb, h)] = att.tile(
                        [65, 512], f32, tag="ctx", bufs=2,
                        name=f"pc{i}_{h}")
                nc.tensor.matmul(
                    ctx_ps[(b, qb, h)],
                    vaug_sb[:, b, kt, h, :],
                    et[:, h * 512:(h + 1) * 512],
                    start=(kt == 0), stop=(kt == NKT - 1))

        # ---- norm chains (staged across slots) ----
        norm_stage = []

        def norm_start(b, qb):
            for h in range(HPC):
                cu = small.tile([65, 512], f32, tag="cu",
                                name=f"cu{b}{qb}{h}")
                nc.vector.tensor_copy(cu, ctx_ps.pop((b, qb, h)))
                norm_stage.append({"b": b, "qb": qb, "h": h, "cu": cu,
                                   "step": 0})

        def norm_advance():
            if not norm_stage:
                return False
            st = norm_stage[0]
            b, qb, h, cu = st["b"], st["qb"], st["h"], st["cu"]
            if st["step"] == 0:
                # reciprocal_approx_fast misbehaves on inputs at a nonzero
                # base partition -- stage the rowsum row at partition 0
                rs = small.tile([1, 512], f32, tag="rs",
                                name=f"rs{b}{qb}{h}")
                nc.vector.tensor_copy(rs, cu[64:65, :])
                rcp_f = small.tile([1, 512], f32, tag="rcpf",
                                   name=f"rcpf{b}{qb}{h}")
                nc.vector.reciprocal_approx_fast(rcp_f, rs)
                rcp = small.tile([1, 512], bf16, tag="rcp",
                                 name=f"rcp{b}{qb}{h}")
                nc.vector.tensor_copy(rcp, rcp_f)
                if debug_dumps:
                    nc.sync.dma_start(dbg_norm_d[b, qb, h, 0], cu[64:65, :])
                    nc.sync.dma_start(dbg_norm_d[b, qb, h, 1], rcp_f)
                st["rcp"] = rcp
                st["step"] = 1
            else:
                pb = att.tile([64, 512], f32, tag="pp", bufs=2,
                              name=f"pb{b}{qb}{h}")
                nc.tensor.matmul(pb, ones_b, st["rcp"], start=True, stop=True)
                nc.vector.tensor_mul(
                    ctx2t_sb[h * 64:(h + 1) * 64, b,
                             qb * 512:(qb + 1) * 512],
                    cu[0:64, :], pb)
                norm_stage.pop(0)
            return True

        # ---- out-projection (deferred, metered) --------------------------
        op_queue = []

        def emit_outproj_unit():
            if not op_queue:
                return False
            b, qb, tt = op_queue.pop(0)
            tok0 = qb * 512 + tt * 128
            ysb = ypool.tile([128, 1024], bf16, tag="ysb",
                             name=f"ysb{b}{qb}{tt}")
            for wh in range(2):
                py = att.tile([128, 512], f32, tag="pp", bufs=2,
                              name=f"py{b}{qb}{tt}{wh}")
                nc.tensor.matmul(
                    py, ctx2t_sb[:, b, tok0:tok0 + 128],
                    wo_sb[:, wh * 512:(wh + 1) * 512],
                    start=True, stop=True)
                nc.vector.tensor_copy(ysb[:, wh * 512:(wh + 1) * 512], py)
            nc.sync.dma_start(
                yp_d[b * S + tok0:b * S + tok0 + 128, :], ysb)
            return True

        # ---- batch-1 QKV: atomic 1-bank closures -------------------------
        # K first (gates batch-1 scores), then V (gates transposes), then Q
        qkv1_pending = []
        vtrans_pending = []

        def queue_qkv1_tile(tb, p3, half):
            # two half-contraction accumulators on separate banks (fast
            # alternating-bank issue), combined by two DVE ops. The two
            # chunks are emitted on consecutive slots; no other pp-tag
            # allocation may occur between them (rotation safety).
            hold = {}

            def chunk_a():
                pq_a = att.tile([128, 512], f32, tag="pp", bufs=2,
                                name=f"pqa_{tb}_{p3}_{half}")
                pq_b = att.tile([128, 512], f32, tag="pp", bufs=2,
                                name=f"pqb_{tb}_{p3}_{half}")
                hold["a"], hold["b"] = pq_a, pq_b
                for a in range(4):
                    nc.tensor.matmul(
                        pq_a if a % 2 == 0 else pq_b,
                        wqkv_sb[:, a, p3 * CF:(p3 + 1) * CF],
                        x2_of[(tb, a)][:, half * 512:(half + 1) * 512],
                        start=(a < 2), stop=False)

            def chunk_b():
                pq_a, pq_b = hold["a"], hold["b"]
                for a in range(4, 8):
                    nc.tensor.matmul(
                        pq_a if a % 2 == 0 else pq_b,
                        wqkv_sb[:, a, p3 * CF:(p3 + 1) * CF],
                        x2_of[(tb, a)][:, half * 512:(half + 1) * 512],
                        start=False, stop=(a >= 6))
                tmp = small.tile([128, 512], bf16, tag="qtmp",
                                 name=f"qtmp{tb}{p3}{half}")
                nc.vector.tensor_scalar_add(tmp, pq_b, btiles[p3])
                dst = qkvt[p3][:, tb * 1024 + half * 512:
                               tb * 1024 + (half + 1) * 512]
                nc.vector.tensor_add(dst, pq_a, tmp)
                if p3 == 2:
                    for k in range(4):
                        vtrans_pending.append(tb * 1024 + half * 512 + k * 128)

            qkv1_pending.append(chunk_a)
            qkv1_pending.append(chunk_b)

        for tb in (2, 3):
            for p3 in (1, 2, 0):        # K, V, Q
                for half in range(2):
                    queue_qkv1_tile(tb, p3, half)

        def emit_vtrans():
            if not vtrans_pending:
                return False
            tok = vtrans_pending.pop(0)
            pv = att.tile([128, 512], f32, tag="pp", bufs=2,
                          name=f"pv1_{tok}")
            nc.tensor.transpose(pv[:, 0:128].bitcast(bf16)[:, 0:128],
                                vt_sb[:, tok:tok + 128], ident_b)
            nc.vector.tensor_copy(
                vaug_sb[:, 1, (tok - S) // 128, :, 0:64],
                pv[:, 0:128].bitcast(bf16)[:, 0:128].rearrange(
                    "p (h d) -> p h d", h=HPC))
            return True

        # ---- the slot loop ----
        group_end_ctx = {16 * g + 15: g for g in range(7)}
        op_gate = []

        for i in range(NSLOT):
            emit_scores(i)
            if i >= 1:
                emit_exp(i - 1)
            if i >= 2:
                j = i - 2
                emit_ctx(j)
                if j in group_end_ctx:
                    gb, gqb, _ = slots[j]
                    norm_start(gb, gqb)
                    op_gate.append((max(i + 4, 64), gb, gqb))
            if i == 16:
                dma_x2(3)
            # batch-1 QKV chunk pairs on slots (4k+3, 4k+4), 3..50
            if i % 4 in (3, 0) and i >= 3 and i <= 50 and qkv1_pending:
                qkv1_pending.pop(0)()
            if i % 4 == 2:
                emit_vtrans()
                emit_vtrans()
            norm_advance()
            while op_gate and op_gate[0][0] <= i:
                _, gb, gqb = op_gate.pop(0)
                for tt in range(4):
                    op_queue.append((gb, gqb, tt))
            if i >= 64 or i % 2 == 1:
                emit_outproj_unit()

        # ---- drain tail ----
        emit_exp(NSLOT - 1)
        emit_ctx(NSLOT - 2)
        emit_ctx(NSLOT - 1)
        norm_start(B - 1, NQB - 1)
        while norm_advance():
            pass
        for tt in range(4):
            op_queue.append((B - 1, NQB - 1, tt))
        while emit_outproj_unit():
            pass

        if debug_dumps:
            for p3 in range(3):
                nc.sync.dma_start(dbg_qkv_d[p3], qkvt[p3])
            nc.sync.dma_start(dbg_vaug_d, vaug_sb)
            nc.sync.dma_start(dbg_ctx2t_d, ctx2t_sb)

    nc.compile()
    return nc


_NC = None


def make_in_maps(x, Wq, bq, Wk, bk, Wv, bv, Wo):
    sc = 1.0 / np.sqrt(np.float32(DK))
    xT = np.ascontiguousarray(
        np.asarray(x, np.float32).reshape(T, F).T).astype(np_bf16)
    in_maps = []
    for c in range(N_CORES):
        sl = slice(c * CF, (c + 1) * CF)
        wqkv = np.concatenate([np.asarray(Wq, np.float32)[:, sl] * sc,
                               np.asarray(Wk, np.float32)[:, sl],
                               np.asarray(Wv, np.float32)[:, sl]], axis=1)
        bqkv = np.concatenate([np.asarray(bq, np.float32)[sl] * sc,
                               np.asarray(bk, np.float32)[sl],
                               np.asarray(bv, np.float32)[sl]])
        in_maps.append({
            "xT": xT,
            "Wqkv": np.ascontiguousarray(wqkv).astype(np_bf16),
            "bqkv": np.ascontiguousarray(bqkv).astype(np.float32),
            "Wo": np.ascontiguousarray(
                np.asarray(Wo, np.float32)[sl, :]).astype(np_bf16),
        })
    return in_maps


def kernel(x, Wq, bq, Wk, bk, Wv, bv, Wo, bo):
    global _NC
    if _NC is None:
        _NC = build_program()
    nc = _NC

    in_maps = make_in_maps(x, Wq, bq, Wk, bk, Wv, bv, Wo)
    res = run_bass_kernel_spmd(nc, in_maps, list(range(N_CORES)))
    y = res.results[0]["yp"].astype(np.float64)
    for c in range(1, N_CORES):
        y += res.results[c]["yp"].astype(np.float64)
    y = (y + np.asarray(bo, np.float64)).astype(np.float32)
    return y.reshape(B, S, F)
